# revision 3
# baseline (speedup 1.0000x reference)
"""Trainium2 Bass kernel for nn_DistillLoss (ragged KL distillation loss).

Strategy (data-parallel over batch, 8 NeuronCores):
  - Host: shard B=1024 samples into 8 x 128. Per core, samples are sorted by
    descending doc count so that, for every doc-slot block, the set of samples
    still needing docs is a prefix of the partitions.
  - Device (per core): the ragged doc segments are fetched straight from the
    core's contiguous doc-row slice with "super-row" indirect DMAs: one index
    per sample gathers BLK consecutive doc rows (BLK*4KB per descriptor) and
    casts f32 -> bf16 inline (SWDGE). Each gather covers only the valid
    partition prefix, so almost no descriptor is wasted.
  - Dot products run on the vector engine in bf16 (2x mode):
    sim[b, m] = sum_d docs[b, m, d] * q[b, d] via scalar_tensor_tensor with
    free-dim accumulate. The 1/TEMP scale is folded into one epilogue op.
  - Masked log-softmax + KL epilogue on [128, 128] f32 tiles; one partial
    scalar per core; host sums and divides by B.
"""

import sys

sys.path.insert(0, "/opt/trn_rl_repo")

import numpy as np

NCORES = 8
B = 1024
D = 1024
M = 128
BL = B // NCORES  # 128 samples per core
TEMP = 0.02
NEG = -1e30
OOB = np.int32(2**30)

_CACHE = {}


def _build_nc(blk=8, rrows=1, cnts=(), bufs=4, use_bf16=True):
    from concourse import bacc, bass, bass_isa, mybir, tile

    f32 = mybir.dt.float32
    bf16 = mybir.dt.bfloat16
    u8 = mybir.dt.uint8
    i32 = mybir.dt.int32
    ALU = mybir.AluOpType
    AF = mybir.ActivationFunctionType
    AX = mybir.AxisListType
    ddt = bf16 if use_bf16 else f32

    nblk = M // blk
    assert len(cnts) == nblk

    nc = bacc.Bacc("TRN2", target_bir_lowering=False, debug=False, num_devices=NCORES)

    rdocs = nc.dram_tensor("rdocs", [rrows, D], f32, kind="ExternalInput").ap()
    idxs = nc.dram_tensor("idxs", [BL, nblk], i32, kind="ExternalInput").ap()
    q = nc.dram_tensor("q", [BL, D], f32, kind="ExternalInput").ap()
    traw = nc.dram_tensor("traw", [BL, M], f32, kind="ExternalInput").ap()
    mask = nc.dram_tensor("mask", [BL, M], u8, kind="ExternalInput").ap()
    out = nc.dram_tensor("out", [1, 1], f32, kind="ExternalOutput").ap()

    from contextlib import ExitStack

    with tile.TileContext(nc) as tc, ExitStack() as ctx:
        consts = ctx.enter_context(tc.tile_pool(name="consts", bufs=1))
        dpool = ctx.enter_context(tc.tile_pool(name="docs", bufs=bufs))
        scratch = ctx.enter_context(tc.tile_pool(name="scratch", bufs=2))
        small = ctx.enter_context(tc.tile_pool(name="small", bufs=1))

        idx_sb = consts.tile([BL, nblk], i32)
        nc.scalar.dma_start(out=idx_sb, in_=idxs)
        traw_sb = consts.tile([BL, M], f32)
        nc.scalar.dma_start(out=traw_sb, in_=traw)
        mask_sb = consts.tile([BL, M], u8)
        nc.scalar.dma_start(out=mask_sb, in_=mask)
        negt = consts.tile([BL, M], f32)
        nc.vector.memset(negt, NEG)

        q_sb = consts.tile([BL, D], ddt)
        if use_bf16:
            nc.gpsimd.dma_start(out=q_sb, in_=q)  # f32 -> bf16 cast in DMA
        else:
            nc.sync.dma_start(out=q_sb, in_=q)

        sim_bm = consts.tile([BL, M], f32)  # raw (unscaled) dots

        for k in range(nblk):
            cnt = int(cnts[k])
            # NOTE: out AP must be 2D — a 3D out tile miscompiles the
            # indirect descriptors (probe3: wrong/partial data lands).
            dtile = dpool.tile([BL, blk * D], ddt)
            if cnt > 0:
                nc.gpsimd.indirect_dma_start(
                    out=dtile[0:cnt, :],
                    out_offset=None,
                    in_=rdocs,
                    in_offset=bass.IndirectOffsetOnAxis(
                        ap=idx_sb[0:cnt, k : k + 1], axis=0
                    ),
                    bounds_check=rrows - 1,
                    oob_is_err=False,
                )
            for j in range(blk):
                m = k * blk + j
                sc = scratch.tile([BL, D], ddt)
                nc.vector.scalar_tensor_tensor(
                    out=sc,
                    in0=dtile[:, j * D : (j + 1) * D],
                    scalar=1.0,
                    in1=q_sb,
                    op0=ALU.mult,
                    op1=ALU.mult,
                    accum_out=sim_bm[:, m : m + 1],
                )

        # ---- epilogue on [b=128, m=128] f32 tiles ----
        simt = small.tile([BL, M], f32)
        nc.vector.tensor_scalar(simt, sim_bm, 1.0 / TEMP, None, op0=ALU.mult)
        simm = small.tile([BL, M], f32)
        nc.vector.select(simm, mask_sb, simt, negt)

        nmx = small.tile([BL, 1], f32)
        nc.vector.tensor_reduce(nmx, simm, axis=AX.X, op=ALU.max, negate=True)
        shifted = small.tile([BL, M], f32)
        nc.vector.tensor_scalar_add(shifted, simm, nmx[:, 0:1])

        e_sb = small.tile([BL, M], f32)
        s_sb = small.tile([BL, 1], f32)
        nc.scalar.activation(e_sb, shifted, AF.Exp, accum_out=s_sb)
        logs = small.tile([BL, 1], f32)
        nc.scalar.activation(logs, s_sb, AF.Ln)

        tsum = small.tile([BL, 1], f32)
        nc.vector.tensor_reduce(tsum, traw_sb, axis=AX.X, op=ALU.add)
        denom = small.tile([BL, 1], f32)
        nc.vector.tensor_scalar_add(denom, tsum, 1e-9)
        rec = small.tile([BL, 1], f32)
        nc.vector.reciprocal(rec, denom)
        tn = small.tile([BL, M], f32)
        nc.vector.tensor_scalar_mul(tn, traw_sb, rec[:, 0:1])
        sumtn = small.tile([BL, 1], f32)
        nc.vector.tensor_mul(sumtn, tsum, rec)

        iszero = small.tile([BL, M], f32)
        nc.vector.tensor_scalar(iszero, tn, 0.0, None, op0=ALU.is_le)
        tsafe = small.tile([BL, M], f32)
        nc.vector.tensor_add(tsafe, tn, iszero)
        logt = small.tile([BL, M], f32)
        nc.scalar.activation(logt, tsafe, AF.Ln)

        sc2 = small.tile([BL, M], f32)
        term1 = small.tile([BL, 1], f32)
        nc.vector.scalar_tensor_tensor(
            out=sc2, in0=tn, scalar=1.0, in1=logt,
            op0=ALU.mult, op1=ALU.mult, accum_out=term1,
        )
        sc3 = small.tile([BL, M], f32)
        term2 = small.tile([BL, 1], f32)
        nc.vector.scalar_tensor_tensor(
            out=sc3, in0=tn, scalar=1.0, in1=shifted,
            op0=ALU.mult, op1=ALU.mult, accum_out=term2,
        )

        lgs = small.tile([BL, 1], f32)
        nc.vector.tensor_mul(lgs, logs, sumtn)
        kc = small.tile([BL, 1], f32)
        nc.vector.tensor_sub(kc, term1, term2)
        nc.vector.tensor_add(kc, kc, lgs)

        tot = small.tile([128, 1], f32)
        nc.gpsimd.partition_all_reduce(
            tot, kc, channels=128, reduce_op=bass_isa.ReduceOp.add
        )
        nc.sync.dma_start(out=out, in_=tot[0:1, 0:1])

    nc.compile()
    return nc


def _get_nc(**cfg):
    key = ("nc",) + tuple(sorted(cfg.items()))
    if key not in _CACHE:
        _CACHE[key] = _build_nc(**cfg)
    return _CACHE[key]


def _make_in_maps(query_embeds, doc_embeds, soft_labels, num_docs_per_sample,
                  blk=8, bufs=4, use_bf16=True):
    qf = np.ascontiguousarray(np.asarray(query_embeds, dtype=np.float32))
    de = np.ascontiguousarray(np.asarray(doc_embeds, dtype=np.float32))
    sl = np.ascontiguousarray(np.asarray(soft_labels, dtype=np.float32))
    nd = np.asarray(num_docs_per_sample).astype(np.int64)
    total = de.shape[0]
    nblk = M // blk

    offs = np.zeros(B, np.int64)
    offs[1:] = np.cumsum(nd)[:-1]
    # effective (clipped) doc counts, mirroring the reference's clip behaviour
    nde = np.minimum(np.minimum(nd, M), np.maximum(total - offs, 0))
    mask = (np.arange(M)[None, :] < nde[:, None]).astype(np.float32)
    traw = sl * mask

    # per-core contiguous doc-row slices
    base = np.empty(NCORES, np.int64)
    rows = np.empty(NCORES, np.int64)
    for c in range(NCORES):
        s0, s1 = c * BL, (c + 1) * BL - 1
        base[c] = offs[s0]
        rows[c] = offs[s1] + nde[s1] - base[c]
    rrows = int(rows.max()) + blk  # blk rows of zero padding for overreads

    # sort samples within each core by descending doc count -> prefix validity
    perm = np.empty(B, np.int64)
    for c in range(NCORES):
        s = slice(c * BL, (c + 1) * BL)
        perm[s] = c * BL + np.argsort(-nde[s], kind="stable")

    # block-start indices into the core slice; OOB sentinel when block invalid
    pos_blk = np.arange(nblk, dtype=np.int64)[None, :] * blk  # [1, nblk]
    relp = (offs[perm] - np.repeat(base, BL))[:, None] + pos_blk  # [B, nblk]
    valid = pos_blk < nde[perm][:, None]
    idx_all = np.where(valid, relp, OOB).astype(np.int32)

    # per-block partition prefix counts, maxed across cores (one SPMD program)
    vcounts = valid.reshape(NCORES, BL, nblk).sum(axis=1)  # [NCORES, nblk]
    cnts = tuple(int(x) for x in vcounts.max(axis=0))

    in_maps = []
    for c in range(NCORES):
        s = slice(c * BL, (c + 1) * BL)
        p = perm[s]
        docs_c = np.zeros((rrows, D), np.float32)
        docs_c[: rows[c]] = de[base[c] : base[c] + rows[c]]
        in_maps.append(
            {
                "rdocs": docs_c,
                "idxs": np.ascontiguousarray(idx_all[s]),
                "q": np.ascontiguousarray(qf[p]),
                "traw": np.ascontiguousarray(traw[p]),
                "mask": np.ascontiguousarray(mask[p].astype(np.uint8)),
            }
        )
    cfg = {"blk": blk, "rrows": rrows, "cnts": cnts, "bufs": bufs,
           "use_bf16": use_bf16}
    return in_maps, cfg


def run(in_maps, cfg=None, trace=False):
    from concourse import bass_utils

    nc = _get_nc(**(cfg or {}))
    return bass_utils.run_bass_kernel_spmd(
        nc, in_maps, list(range(NCORES)), trace=trace
    )


def kernel(query_embeds, doc_embeds, soft_labels, num_docs_per_sample):
    in_maps, cfg = _make_in_maps(
        query_embeds, doc_embeds, soft_labels, num_docs_per_sample
    )
    res = run(in_maps, cfg=cfg)
    tot = sum(float(r["out"][0, 0]) for r in res.results)
    return np.asarray(tot / B, dtype=np.float32)


# revision 10
# speedup vs baseline: 1.1713x; 1.1713x over previous
"""Trainium2 Bass kernel for nn_DistillLoss (ragged KL distillation loss).

Strategy (data-parallel over batch, 8 NeuronCores):
  - Host: shard B=1024 samples into 8 x 128. Per core, samples are sorted by
    descending doc count so that, for every doc-slot block, the set of samples
    still needing docs is a prefix of the partitions.
  - Device (per core): the ragged doc segments are fetched straight from the
    core's contiguous doc-row slice with "super-row" indirect DMAs: one index
    per sample gathers BLK consecutive doc rows (BLK*4KB per descriptor) and
    casts f32 -> bf16 inline (SWDGE). Each gather covers only the valid
    partition prefix, so almost no descriptor is wasted.
  - Dot products run on the vector engine in bf16 (2x mode):
    sim[b, m] = sum_d docs[b, m, d] * q[b, d] via scalar_tensor_tensor with
    free-dim accumulate. The 1/TEMP scale is folded into one epilogue op.
  - Masked log-softmax + KL epilogue on [128, 128] f32 tiles; one partial
    scalar per core; host sums and divides by B.
"""

import sys

sys.path.insert(0, "/opt/trn_rl_repo")

import numpy as np

NCORES = 8
B = 1024
D = 1024
M = 128
BL = B // NCORES  # 128 samples per core
TEMP = 0.02
NEG = -1e30
OOB = np.int32(2**30)

_CACHE = {}


def _build_nc(blk=8, rrows=1, bufs=6, use_bf16=True):
    from concourse import bacc, bass, bass_isa, mybir, tile

    f32 = mybir.dt.float32
    bf16 = mybir.dt.bfloat16
    u8 = mybir.dt.uint8
    i32 = mybir.dt.int32
    ALU = mybir.AluOpType
    AF = mybir.ActivationFunctionType
    AX = mybir.AxisListType
    ddt = bf16 if use_bf16 else f32

    nblk = M // blk

    nc = bacc.Bacc("TRN2", target_bir_lowering=False, debug=False, num_devices=NCORES)

    rdocs = nc.dram_tensor("rdocs", [rrows, D], f32, kind="ExternalInput").ap()
    idxs = nc.dram_tensor("idxs", [BL, nblk], i32, kind="ExternalInput").ap()
    q = nc.dram_tensor("q", [BL, D], f32, kind="ExternalInput").ap()
    traw = nc.dram_tensor("traw", [BL, M], f32, kind="ExternalInput").ap()
    mask = nc.dram_tensor("mask", [BL, M], u8, kind="ExternalInput").ap()
    out = nc.dram_tensor("out", [1, 1], f32, kind="ExternalOutput").ap()

    from contextlib import ExitStack

    with tile.TileContext(nc) as tc, ExitStack() as ctx:
        consts = ctx.enter_context(tc.tile_pool(name="consts", bufs=1))
        dpool = ctx.enter_context(tc.tile_pool(name="docs", bufs=bufs))
        scratch = ctx.enter_context(tc.tile_pool(name="scratch", bufs=4))
        sc2_act = ctx.enter_context(tc.tile_pool(name="actout", bufs=2))
        small = ctx.enter_context(tc.tile_pool(name="small", bufs=1))

        idx_sb = consts.tile([BL, nblk], i32)
        nc.scalar.dma_start(out=idx_sb, in_=idxs)
        traw_sb = consts.tile([BL, M], f32)
        nc.scalar.dma_start(out=traw_sb, in_=traw)
        mask_sb = consts.tile([BL, M], u8)
        nc.scalar.dma_start(out=mask_sb, in_=mask)
        negt = consts.tile([BL, M], f32)
        nc.vector.memset(negt, NEG)

        q_sb = consts.tile([BL, D], ddt)
        if use_bf16:
            nc.gpsimd.dma_start(out=q_sb, in_=q)  # f32 -> bf16 cast in DMA
        else:
            nc.sync.dma_start(out=q_sb, in_=q)

        sim_bm = consts.tile([BL, M], f32)  # raw (unscaled) dots

        for k in range(nblk):
            # NOTE: out AP must be 2D — a 3D out tile miscompiles the
            # indirect descriptors (probe3: wrong/partial data lands).
            # Full 128-partition gathers keep descriptors balanced across
            # the 16 SDMA engines; OOB-skipped descriptors are ~free.
            dtile = dpool.tile([BL, blk * D], ddt)
            nc.gpsimd.indirect_dma_start(
                out=dtile,
                out_offset=None,
                in_=rdocs,
                in_offset=bass.IndirectOffsetOnAxis(
                    ap=idx_sb[:, k : k + 1], axis=0
                ),
                bounds_check=rrows - 1,
                oob_is_err=False,
            )
            for j in range(blk):
                m = k * blk + j
                # DVE bf16 tensor_tensor multiply runs in 2x mode; the
                # free-dim accumulate runs on the otherwise-idle Scalar
                # engine (scalar_tensor_tensor's accum path is 1x-only).
                sc = scratch.tile([BL, D], ddt)
                nc.vector.tensor_mul(sc, dtile[:, j * D : (j + 1) * D], q_sb)
                aout = sc2_act.tile([BL, D], ddt, tag="aout")
                nc.scalar.activation(
                    aout, sc, AF.Copy, accum_out=sim_bm[:, m : m + 1]
                )

        # ---- epilogue on [b=128, m=128] f32 tiles ----
        simt = small.tile([BL, M], f32)
        nc.vector.tensor_scalar(simt, sim_bm, 1.0 / TEMP, None, op0=ALU.mult)
        simm = small.tile([BL, M], f32)
        nc.vector.select(simm, mask_sb, simt, negt)

        nmx = small.tile([BL, 1], f32)
        nc.vector.tensor_reduce(nmx, simm, axis=AX.X, op=ALU.max, negate=True)
        shifted = small.tile([BL, M], f32)
        nc.vector.tensor_scalar_add(shifted, simm, nmx[:, 0:1])

        e_sb = small.tile([BL, M], f32)
        s_sb = small.tile([BL, 1], f32)
        nc.scalar.activation(e_sb, shifted, AF.Exp, accum_out=s_sb)
        logs = small.tile([BL, 1], f32)
        nc.scalar.activation(logs, s_sb, AF.Ln)

        tsum = small.tile([BL, 1], f32)
        nc.vector.tensor_reduce(tsum, traw_sb, axis=AX.X, op=ALU.add)
        denom = small.tile([BL, 1], f32)
        nc.vector.tensor_scalar_add(denom, tsum, 1e-9)
        rec = small.tile([BL, 1], f32)
        nc.vector.reciprocal(rec, denom)
        tn = small.tile([BL, M], f32)
        nc.vector.tensor_scalar_mul(tn, traw_sb, rec[:, 0:1])
        sumtn = small.tile([BL, 1], f32)
        nc.vector.tensor_mul(sumtn, tsum, rec)

        iszero = small.tile([BL, M], f32)
        nc.vector.tensor_scalar(iszero, tn, 0.0, None, op0=ALU.is_le)
        tsafe = small.tile([BL, M], f32)
        nc.vector.tensor_add(tsafe, tn, iszero)
        logt = small.tile([BL, M], f32)
        nc.scalar.activation(logt, tsafe, AF.Ln)

        sc2 = small.tile([BL, M], f32)
        term1 = small.tile([BL, 1], f32)
        nc.vector.scalar_tensor_tensor(
            out=sc2, in0=tn, scalar=1.0, in1=logt,
            op0=ALU.mult, op1=ALU.mult, accum_out=term1,
        )
        sc3 = small.tile([BL, M], f32)
        term2 = small.tile([BL, 1], f32)
        nc.vector.scalar_tensor_tensor(
            out=sc3, in0=tn, scalar=1.0, in1=shifted,
            op0=ALU.mult, op1=ALU.mult, accum_out=term2,
        )

        lgs = small.tile([BL, 1], f32)
        nc.vector.tensor_mul(lgs, logs, sumtn)
        kc = small.tile([BL, 1], f32)
        nc.vector.tensor_sub(kc, term1, term2)
        nc.vector.tensor_add(kc, kc, lgs)

        tot = small.tile([128, 1], f32)
        nc.gpsimd.partition_all_reduce(
            tot, kc, channels=128, reduce_op=bass_isa.ReduceOp.add
        )
        nc.sync.dma_start(out=out, in_=tot[0:1, 0:1])

    nc.compile()
    return nc


def _get_nc(**cfg):
    key = ("nc",) + tuple(sorted(cfg.items()))
    if key not in _CACHE:
        _CACHE[key] = _build_nc(**cfg)
    return _CACHE[key]


def _make_in_maps(query_embeds, doc_embeds, soft_labels, num_docs_per_sample,
                  blk=8, bufs=6, use_bf16=True):
    qf = np.ascontiguousarray(np.asarray(query_embeds, dtype=np.float32))
    de = np.ascontiguousarray(np.asarray(doc_embeds, dtype=np.float32))
    sl = np.ascontiguousarray(np.asarray(soft_labels, dtype=np.float32))
    nd = np.asarray(num_docs_per_sample).astype(np.int64)
    total = de.shape[0]
    nblk = M // blk

    offs = np.zeros(B, np.int64)
    offs[1:] = np.cumsum(nd)[:-1]
    # effective (clipped) doc counts, mirroring the reference's clip behaviour
    nde = np.minimum(np.minimum(nd, M), np.maximum(total - offs, 0))
    mask = (np.arange(M)[None, :] < nde[:, None]).astype(np.float32)
    traw = sl * mask

    # per-core contiguous doc-row slices
    base = np.empty(NCORES, np.int64)
    rows = np.empty(NCORES, np.int64)
    for c in range(NCORES):
        s0, s1 = c * BL, (c + 1) * BL - 1
        base[c] = offs[s0]
        rows[c] = offs[s1] + nde[s1] - base[c]
    rrows = int(rows.max()) + blk  # blk rows of zero padding for overreads

    # block-start indices into the core slice; OOB sentinel when block invalid
    pos_blk = np.arange(nblk, dtype=np.int64)[None, :] * blk  # [1, nblk]
    relp = (offs - np.repeat(base, BL))[:, None] + pos_blk  # [B, nblk]
    valid = pos_blk < nde[:, None]
    idx_all = np.where(valid, relp, OOB).astype(np.int32)

    in_maps = []
    for c in range(NCORES):
        s = slice(c * BL, (c + 1) * BL)
        docs_c = np.zeros((rrows, D), np.float32)
        docs_c[: rows[c]] = de[base[c] : base[c] + rows[c]]
        in_maps.append(
            {
                "rdocs": docs_c,
                "idxs": np.ascontiguousarray(idx_all[s]),
                "q": np.ascontiguousarray(qf[s]),
                "traw": np.ascontiguousarray(traw[s]),
                "mask": np.ascontiguousarray(mask[s].astype(np.uint8)),
            }
        )
    cfg = {"blk": blk, "rrows": rrows, "bufs": bufs, "use_bf16": use_bf16}
    return in_maps, cfg


def run(in_maps, cfg=None, trace=False):
    from concourse import bass_utils

    nc = _get_nc(**(cfg or {}))
    return bass_utils.run_bass_kernel_spmd(
        nc, in_maps, list(range(NCORES)), trace=trace
    )


def kernel(query_embeds, doc_embeds, soft_labels, num_docs_per_sample):
    in_maps, cfg = _make_in_maps(
        query_embeds, doc_embeds, soft_labels, num_docs_per_sample
    )
    res = run(in_maps, cfg=cfg)
    tot = sum(float(r["out"][0, 0]) for r in res.results)
    return np.asarray(tot / B, dtype=np.float32)


# revision 16
# speedup vs baseline: 1.2633x; 1.0785x over previous
"""Trainium2 Bass kernel for nn_DistillLoss (ragged KL distillation loss).

Strategy (data-parallel over batch, 8 NeuronCores):
  - Host: shard B=1024 samples into 8 x 128. Per core, samples are sorted by
    descending doc count so that, for every doc-slot block, the set of samples
    still needing docs is a prefix of the partitions.
  - Device (per core): the ragged doc segments are fetched straight from the
    core's contiguous doc-row slice with "super-row" indirect DMAs: one index
    per sample gathers BLK consecutive doc rows (BLK*4KB per descriptor) and
    casts f32 -> bf16 inline (SWDGE). Each gather covers only the valid
    partition prefix, so almost no descriptor is wasted.
  - Dot products run on the vector engine in bf16 (2x mode):
    sim[b, m] = sum_d docs[b, m, d] * q[b, d] via scalar_tensor_tensor with
    free-dim accumulate. The 1/TEMP scale is folded into one epilogue op.
  - Masked log-softmax + KL epilogue on [128, 128] f32 tiles; one partial
    scalar per core; host sums and divides by B.
"""

import sys

sys.path.insert(0, "/opt/trn_rl_repo")

import numpy as np

NCORES = 8
B = 1024
D = 1024
M = 128
BL = B // NCORES  # 128 samples per core
TEMP = 0.02
NEG = -1e30
OOB = np.int32(2**30)

_CACHE = {}


def _build_nc(blk=8, rrows=1, bufs=6, use_bf16=True, act_slots=5):
    from concourse import bacc, bass, bass_isa, mybir, tile

    f32 = mybir.dt.float32
    bf16 = mybir.dt.bfloat16
    u8 = mybir.dt.uint8
    i32 = mybir.dt.int32
    ALU = mybir.AluOpType
    AF = mybir.ActivationFunctionType
    AX = mybir.AxisListType
    ddt = bf16 if use_bf16 else f32

    nblk = M // blk

    nc = bacc.Bacc("TRN2", target_bir_lowering=False, debug=False, num_devices=NCORES)

    rdocs = nc.dram_tensor("rdocs", [rrows, D], f32, kind="ExternalInput").ap()
    idxs = nc.dram_tensor("idxs", [BL, nblk], i32, kind="ExternalInput").ap()
    q = nc.dram_tensor("q", [BL, D], f32, kind="ExternalInput").ap()
    traw = nc.dram_tensor("traw", [BL, M], f32, kind="ExternalInput").ap()
    mask = nc.dram_tensor("mask", [BL, M], u8, kind="ExternalInput").ap()
    out = nc.dram_tensor("out", [1, 1], f32, kind="ExternalOutput").ap()

    from contextlib import ExitStack

    with tile.TileContext(nc) as tc, ExitStack() as ctx:
        consts = ctx.enter_context(tc.tile_pool(name="consts", bufs=1))
        dpool = ctx.enter_context(tc.tile_pool(name="docs", bufs=bufs))
        scratch = ctx.enter_context(tc.tile_pool(name="scratch", bufs=3))
        sc2_act = ctx.enter_context(tc.tile_pool(name="actout", bufs=2))
        small = ctx.enter_context(tc.tile_pool(name="small", bufs=1))

        idx_sb = consts.tile([BL, nblk], i32)
        nc.scalar.dma_start(out=idx_sb, in_=idxs)
        traw_sb = consts.tile([BL, M], f32)
        nc.scalar.dma_start(out=traw_sb, in_=traw)
        mask_sb = consts.tile([BL, M], u8)
        nc.scalar.dma_start(out=mask_sb, in_=mask)
        negt = consts.tile([BL, M], f32)
        nc.vector.memset(negt, NEG)

        q_sb = consts.tile([BL, D], ddt)
        if use_bf16:
            nc.gpsimd.dma_start(out=q_sb, in_=q)  # f32 -> bf16 cast in DMA
        else:
            nc.sync.dma_start(out=q_sb, in_=q)

        # q replicated act_slots times for the wide per-block multiply
        qrep = consts.tile([BL, act_slots * D], ddt)
        for r in range(act_slots):
            nc.vector.tensor_copy(qrep[:, r * D : (r + 1) * D], q_sb)

        sim_bm = consts.tile([BL, M], f32)  # raw (unscaled) dots

        for k in range(nblk):
            # NOTE: out AP must be 2D — a 3D out tile miscompiles the
            # indirect descriptors (probe3: wrong/partial data lands).
            # Full 128-partition gathers keep descriptors balanced across
            # the 16 SDMA engines; OOB-skipped descriptors are ~free.
            dtile = dpool.tile([BL, blk * D], ddt)
            nc.gpsimd.indirect_dma_start(
                out=dtile,
                out_offset=None,
                in_=rdocs,
                in_offset=bass.IndirectOffsetOnAxis(
                    ap=idx_sb[:, k : k + 1], axis=0
                ),
                bounds_check=rrows - 1,
                oob_is_err=False,
            )
            # Work split per block of `blk` doc slots: the first `act_slots`
            # are multiplied in one wide DVE tensor_tensor (bf16 2x mode)
            # and accumulated on the Scalar engine (activation Copy with
            # free-dim accum); the rest run fused on DVE via
            # scalar_tensor_tensor (1x, but single pass).
            sc = scratch.tile([BL, act_slots * D], ddt)
            nc.vector.tensor_mul(sc, dtile[:, : act_slots * D], qrep)
            for j in range(act_slots):
                m = k * blk + j
                aout = sc2_act.tile([BL, D], ddt, tag="aout")
                nc.scalar.activation(
                    aout, sc[:, j * D : (j + 1) * D], AF.Copy,
                    accum_out=sim_bm[:, m : m + 1],
                )
            for j in range(act_slots, blk):
                m = k * blk + j
                scf = scratch.tile([BL, D], ddt, tag="scf")
                nc.vector.scalar_tensor_tensor(
                    out=scf,
                    in0=dtile[:, j * D : (j + 1) * D],
                    scalar=1.0,
                    in1=q_sb,
                    op0=ALU.mult,
                    op1=ALU.mult,
                    accum_out=sim_bm[:, m : m + 1],
                )

        # ---- epilogue on [b=128, m=128] f32 tiles ----
        simt = small.tile([BL, M], f32)
        nc.vector.tensor_scalar(simt, sim_bm, 1.0 / TEMP, None, op0=ALU.mult)
        simm = small.tile([BL, M], f32)
        nc.vector.select(simm, mask_sb, simt, negt)

        nmx = small.tile([BL, 1], f32)
        nc.vector.tensor_reduce(nmx, simm, axis=AX.X, op=ALU.max, negate=True)
        shifted = small.tile([BL, M], f32)
        nc.vector.tensor_scalar_add(shifted, simm, nmx[:, 0:1])

        e_sb = small.tile([BL, M], f32)
        s_sb = small.tile([BL, 1], f32)
        nc.scalar.activation(e_sb, shifted, AF.Exp, accum_out=s_sb)
        logs = small.tile([BL, 1], f32)
        nc.scalar.activation(logs, s_sb, AF.Ln)

        tsum = small.tile([BL, 1], f32)
        nc.vector.tensor_reduce(tsum, traw_sb, axis=AX.X, op=ALU.add)
        denom = small.tile([BL, 1], f32)
        nc.vector.tensor_scalar_add(denom, tsum, 1e-9)
        rec = small.tile([BL, 1], f32)
        nc.vector.reciprocal(rec, denom)
        tn = small.tile([BL, M], f32)
        nc.vector.tensor_scalar_mul(tn, traw_sb, rec[:, 0:1])
        sumtn = small.tile([BL, 1], f32)
        nc.vector.tensor_mul(sumtn, tsum, rec)

        iszero = small.tile([BL, M], f32)
        nc.vector.tensor_scalar(iszero, tn, 0.0, None, op0=ALU.is_le)
        tsafe = small.tile([BL, M], f32)
        nc.vector.tensor_add(tsafe, tn, iszero)
        logt = small.tile([BL, M], f32)
        nc.scalar.activation(logt, tsafe, AF.Ln)

        sc2 = small.tile([BL, M], f32)
        term1 = small.tile([BL, 1], f32)
        nc.vector.scalar_tensor_tensor(
            out=sc2, in0=tn, scalar=1.0, in1=logt,
            op0=ALU.mult, op1=ALU.mult, accum_out=term1,
        )
        sc3 = small.tile([BL, M], f32)
        term2 = small.tile([BL, 1], f32)
        nc.vector.scalar_tensor_tensor(
            out=sc3, in0=tn, scalar=1.0, in1=shifted,
            op0=ALU.mult, op1=ALU.mult, accum_out=term2,
        )

        lgs = small.tile([BL, 1], f32)
        nc.vector.tensor_mul(lgs, logs, sumtn)
        kc = small.tile([BL, 1], f32)
        nc.vector.tensor_sub(kc, term1, term2)
        nc.vector.tensor_add(kc, kc, lgs)

        tot = small.tile([128, 1], f32)
        nc.gpsimd.partition_all_reduce(
            tot, kc, channels=128, reduce_op=bass_isa.ReduceOp.add
        )
        nc.sync.dma_start(out=out, in_=tot[0:1, 0:1])

    nc.compile()
    return nc


def _get_nc(**cfg):
    key = ("nc",) + tuple(sorted(cfg.items()))
    if key not in _CACHE:
        _CACHE[key] = _build_nc(**cfg)
    return _CACHE[key]


def _make_in_maps(query_embeds, doc_embeds, soft_labels, num_docs_per_sample,
                  blk=8, bufs=6, use_bf16=True, act_slots=5):
    qf = np.ascontiguousarray(np.asarray(query_embeds, dtype=np.float32))
    de = np.ascontiguousarray(np.asarray(doc_embeds, dtype=np.float32))
    sl = np.ascontiguousarray(np.asarray(soft_labels, dtype=np.float32))
    nd = np.asarray(num_docs_per_sample).astype(np.int64)
    total = de.shape[0]
    nblk = M // blk

    offs = np.zeros(B, np.int64)
    offs[1:] = np.cumsum(nd)[:-1]
    # effective (clipped) doc counts, mirroring the reference's clip behaviour
    nde = np.minimum(np.minimum(nd, M), np.maximum(total - offs, 0))
    mask = (np.arange(M)[None, :] < nde[:, None]).astype(np.float32)
    traw = sl * mask

    # per-core contiguous doc-row slices
    base = np.empty(NCORES, np.int64)
    rows = np.empty(NCORES, np.int64)
    for c in range(NCORES):
        s0, s1 = c * BL, (c + 1) * BL - 1
        base[c] = offs[s0]
        rows[c] = offs[s1] + nde[s1] - base[c]
    rrows = int(rows.max()) + blk  # blk rows of zero padding for overreads

    # block-start indices into the core slice; OOB sentinel when block invalid
    pos_blk = np.arange(nblk, dtype=np.int64)[None, :] * blk  # [1, nblk]
    relp = (offs - np.repeat(base, BL))[:, None] + pos_blk  # [B, nblk]
    valid = pos_blk < nde[:, None]
    idx_all = np.where(valid, relp, OOB).astype(np.int32)

    in_maps = []
    for c in range(NCORES):
        s = slice(c * BL, (c + 1) * BL)
        docs_c = np.zeros((rrows, D), np.float32)
        docs_c[: rows[c]] = de[base[c] : base[c] + rows[c]]
        in_maps.append(
            {
                "rdocs": docs_c,
                "idxs": np.ascontiguousarray(idx_all[s]),
                "q": np.ascontiguousarray(qf[s]),
                "traw": np.ascontiguousarray(traw[s]),
                "mask": np.ascontiguousarray(mask[s].astype(np.uint8)),
            }
        )
    cfg = {"blk": blk, "rrows": rrows, "bufs": bufs, "use_bf16": use_bf16,
           "act_slots": act_slots}
    return in_maps, cfg


def run(in_maps, cfg=None, trace=False):
    from concourse import bass_utils

    nc = _get_nc(**(cfg or {}))
    return bass_utils.run_bass_kernel_spmd(
        nc, in_maps, list(range(NCORES)), trace=trace
    )


def kernel(query_embeds, doc_embeds, soft_labels, num_docs_per_sample):
    in_maps, cfg = _make_in_maps(
        query_embeds, doc_embeds, soft_labels, num_docs_per_sample
    )
    res = run(in_maps, cfg=cfg)
    tot = sum(float(r["out"][0, 0]) for r in res.results)
    return np.asarray(tot / B, dtype=np.float32)


# revision 25
# speedup vs baseline: 1.5139x; 1.1985x over previous
"""Trainium2 Bass kernel for nn_DistillLoss (ragged KL distillation loss).

Strategy (data-parallel over batch, 8 NeuronCores):
  - Host: shard B=1024 samples into 8 x 128. Per core, samples are sorted by
    descending doc count so that, for every doc-slot block, the set of samples
    still needing docs is a prefix of the partitions.
  - Device (per core): the ragged doc segments are fetched straight from the
    core's contiguous doc-row slice with "super-row" indirect DMAs: one index
    per sample gathers BLK consecutive doc rows (BLK*4KB per descriptor) and
    casts f32 -> bf16 inline (SWDGE). Each gather covers only the valid
    partition prefix, so almost no descriptor is wasted.
  - Dot products run on the vector engine in bf16 (2x mode):
    sim[b, m] = sum_d docs[b, m, d] * q[b, d] via scalar_tensor_tensor with
    free-dim accumulate. The 1/TEMP scale is folded into one epilogue op.
  - Masked log-softmax + KL epilogue on [128, 128] f32 tiles; one partial
    scalar per core; host sums and divides by B.
"""

import sys

sys.path.insert(0, "/opt/trn_rl_repo")

import numpy as np

NCORES = 8
B = 1024
D = 1024
M = 128
BL = B // NCORES  # 128 samples per core
TEMP = 0.02
NEG = -1e30
OOB = np.int32(2**30)

_CACHE = {}


def _build_nc(blk=8, rrows=1, bufs=6, use_bf16=True, act_slots=5, gps_slots=0):
    from concourse import bacc, bass, bass_isa, mybir, tile

    f32 = mybir.dt.float32
    bf16 = mybir.dt.bfloat16
    u8 = mybir.dt.uint8
    i32 = mybir.dt.int32
    ALU = mybir.AluOpType
    AF = mybir.ActivationFunctionType
    AX = mybir.AxisListType
    ddt = bf16 if use_bf16 else f32

    nblk = M // blk

    nc = bacc.Bacc("TRN2", target_bir_lowering=False, debug=False, num_devices=NCORES)

    rdocs = nc.dram_tensor("rdocs", [rrows, D], f32, kind="ExternalInput").ap()
    idxs = nc.dram_tensor("idxs", [BL, nblk], i32, kind="ExternalInput").ap()
    q = nc.dram_tensor("q", [BL, D], f32, kind="ExternalInput").ap()
    traw = nc.dram_tensor("traw", [BL, M], f32, kind="ExternalInput").ap()
    mask = nc.dram_tensor("mask", [BL, M], u8, kind="ExternalInput").ap()
    out = nc.dram_tensor("out", [1, 1], f32, kind="ExternalOutput").ap()

    from contextlib import ExitStack

    with tile.TileContext(nc) as tc, ExitStack() as ctx:
        consts = ctx.enter_context(tc.tile_pool(name="consts", bufs=1))
        dpool = ctx.enter_context(tc.tile_pool(name="docs", bufs=bufs))
        scratch = ctx.enter_context(tc.tile_pool(name="scratch", bufs=3))
        sc2_act = ctx.enter_context(tc.tile_pool(name="actout", bufs=2))
        small = ctx.enter_context(tc.tile_pool(name="small", bufs=1))

        idx_sb = consts.tile([BL, nblk], i32)
        nc.sync.dma_start(out=idx_sb, in_=idxs)
        traw_sb = consts.tile([BL, M], f32)
        nc.scalar.dma_start(out=traw_sb, in_=traw)
        mask_sb = consts.tile([BL, M], u8)
        nc.scalar.dma_start(out=mask_sb, in_=mask)
        negt = consts.tile([BL, M], f32)
        nc.vector.memset(negt, NEG)

        # q loads f32 via HWDGE; cast to bf16 on DVE so the Q7/SWDGE path
        # has nothing to do before the first gather's descriptors.
        qf_sb = consts.tile([BL, D], f32)
        nc.scalar.dma_start(out=qf_sb, in_=q)
        q_sb = consts.tile([BL, D], ddt)
        nc.vector.tensor_copy(q_sb, qf_sb)

        # q replicated act_slots times for the wide per-block multiply
        qrep = consts.tile([BL, act_slots * D], ddt)
        for r in range(act_slots):
            nc.vector.tensor_copy(qrep[:, r * D : (r + 1) * D], q_sb)

        sim_bm = consts.tile([BL, M], f32)  # raw (unscaled) dots

        for k in range(nblk):
            # NOTE: out AP must be 2D — a 3D out tile miscompiles the
            # indirect descriptors (probe3: wrong/partial data lands).
            # Full 128-partition gathers keep descriptors balanced across
            # the 16 SDMA engines; OOB-skipped descriptors are ~free.
            dtile = dpool.tile([BL, blk * D], ddt)
            nc.gpsimd.indirect_dma_start(
                out=dtile,
                out_offset=None,
                in_=rdocs,
                in_offset=bass.IndirectOffsetOnAxis(
                    ap=idx_sb[:, k : k + 1], axis=0
                ),
                bounds_check=rrows - 1,
                oob_is_err=False,
            )
            # Work split per block of `blk` doc slots: the first `act_slots`
            # are multiplied in one wide DVE tensor_tensor (bf16 2x mode)
            # and accumulated on the Scalar engine (activation with free-dim
            # accum); `gps_slots` run fused on GpSimd; the rest run fused on
            # DVE via scalar_tensor_tensor (1x, but single pass).
            sc = scratch.tile([BL, act_slots * D], ddt)
            nc.vector.tensor_mul(sc, dtile[:, : act_slots * D], qrep)
            for j in range(act_slots):
                m = k * blk + j
                aout = sc2_act.tile([BL, D], ddt, tag="aout")
                # NOTE: AF.Identity is a lossy spline approximation (probe4:
                # max rel err 0.53); AF.Copy is exact.
                nc.scalar.activation(
                    aout, sc[:, j * D : (j + 1) * D], AF.Copy,
                    accum_out=sim_bm[:, m : m + 1],
                )
            for j in range(act_slots, blk):
                m = k * blk + j
                on_gps = j >= blk - gps_slots
                eng = nc.gpsimd if on_gps else nc.vector
                scf = scratch.tile([BL, D], ddt, tag="scg" if on_gps else "scf")
                eng.scalar_tensor_tensor(
                    out=scf,
                    in0=dtile[:, j * D : (j + 1) * D],
                    scalar=1.0,
                    in1=q_sb,
                    op0=ALU.mult,
                    op1=ALU.mult,
                    accum_out=sim_bm[:, m : m + 1],
                )

        # ---- epilogue on [b=128, m=128] f32 tiles ----
        simt = small.tile([BL, M], f32)
        nc.vector.tensor_scalar(simt, sim_bm, 1.0 / TEMP, None, op0=ALU.mult)
        simm = small.tile([BL, M], f32)
        nc.vector.select(simm, mask_sb, simt, negt)

        nmx = small.tile([BL, 1], f32)
        nc.vector.tensor_reduce(nmx, simm, axis=AX.X, op=ALU.max, negate=True)
        shifted = small.tile([BL, M], f32)
        nc.vector.tensor_scalar_add(shifted, simm, nmx[:, 0:1])

        e_sb = small.tile([BL, M], f32)
        s_sb = small.tile([BL, 1], f32)
        nc.scalar.activation(e_sb, shifted, AF.Exp, accum_out=s_sb)
        logs = small.tile([BL, 1], f32)
        nc.scalar.activation(logs, s_sb, AF.Ln)

        tsum = small.tile([BL, 1], f32)
        nc.vector.tensor_reduce(tsum, traw_sb, axis=AX.X, op=ALU.add)
        denom = small.tile([BL, 1], f32)
        nc.vector.tensor_scalar_add(denom, tsum, 1e-9)
        rec = small.tile([BL, 1], f32)
        nc.vector.reciprocal(rec, denom)
        tn = small.tile([BL, M], f32)
        nc.vector.tensor_scalar_mul(tn, traw_sb, rec[:, 0:1])
        sumtn = small.tile([BL, 1], f32)
        nc.vector.tensor_mul(sumtn, tsum, rec)

        iszero = small.tile([BL, M], f32)
        nc.vector.tensor_scalar(iszero, tn, 0.0, None, op0=ALU.is_le)
        tsafe = small.tile([BL, M], f32)
        nc.vector.tensor_add(tsafe, tn, iszero)
        logt = small.tile([BL, M], f32)
        nc.scalar.activation(logt, tsafe, AF.Ln)

        sc2 = small.tile([BL, M], f32)
        term1 = small.tile([BL, 1], f32)
        nc.vector.scalar_tensor_tensor(
            out=sc2, in0=tn, scalar=1.0, in1=logt,
            op0=ALU.mult, op1=ALU.mult, accum_out=term1,
        )
        sc3 = small.tile([BL, M], f32)
        term2 = small.tile([BL, 1], f32)
        nc.vector.scalar_tensor_tensor(
            out=sc3, in0=tn, scalar=1.0, in1=shifted,
            op0=ALU.mult, op1=ALU.mult, accum_out=term2,
        )

        lgs = small.tile([BL, 1], f32)
        nc.vector.tensor_mul(lgs, logs, sumtn)
        kc = small.tile([BL, 1], f32)
        nc.vector.tensor_sub(kc, term1, term2)
        nc.vector.tensor_add(kc, kc, lgs)

        tot = small.tile([128, 1], f32)
        nc.gpsimd.partition_all_reduce(
            tot, kc, channels=128, reduce_op=bass_isa.ReduceOp.add
        )
        nc.sync.dma_start(out=out, in_=tot[0:1, 0:1])

    nc.compile()
    return nc


def _get_nc(**cfg):
    key = ("nc",) + tuple(sorted(cfg.items()))
    if key not in _CACHE:
        _CACHE[key] = _build_nc(**cfg)
    return _CACHE[key]


def _make_in_maps(query_embeds, doc_embeds, soft_labels, num_docs_per_sample,
                  blk=8, bufs=6, use_bf16=True, act_slots=5, gps_slots=0):
    qf = np.ascontiguousarray(np.asarray(query_embeds, dtype=np.float32))
    de = np.ascontiguousarray(np.asarray(doc_embeds, dtype=np.float32))
    sl = np.ascontiguousarray(np.asarray(soft_labels, dtype=np.float32))
    nd = np.asarray(num_docs_per_sample).astype(np.int64)
    total = de.shape[0]
    nblk = M // blk

    offs = np.zeros(B, np.int64)
    offs[1:] = np.cumsum(nd)[:-1]
    # effective (clipped) doc counts, mirroring the reference's clip behaviour
    nde = np.minimum(np.minimum(nd, M), np.maximum(total - offs, 0))
    mask = (np.arange(M)[None, :] < nde[:, None]).astype(np.float32)
    traw = sl * mask

    # per-core contiguous doc-row slices
    base = np.empty(NCORES, np.int64)
    rows = np.empty(NCORES, np.int64)
    for c in range(NCORES):
        s0, s1 = c * BL, (c + 1) * BL - 1
        base[c] = offs[s0]
        rows[c] = offs[s1] + nde[s1] - base[c]
    rrows = int(rows.max()) + blk  # blk rows of zero padding for overreads

    # block-start indices into the core slice; OOB sentinel when block invalid
    pos_blk = np.arange(nblk, dtype=np.int64)[None, :] * blk  # [1, nblk]
    relp = (offs - np.repeat(base, BL))[:, None] + pos_blk  # [B, nblk]
    valid = pos_blk < nde[:, None]
    idx_all = np.where(valid, relp, OOB).astype(np.int32)

    # Each indirect-gather descriptor for partition p lands on SDMA engine
    # 2*((p%32)//4) + p//64 (granules of 4 partitions; each engine serves 8
    # partitions). Per-core, permute samples so per-engine bytes are even,
    # giving the known-slow engine 15 a lighter share.
    rate = np.ones(16)
    rate[15] = 0.72
    gran_eng = np.array([2 * (g % 8) + g // 16 for g in range(32)])
    eng_parts = {e: [] for e in range(16)}
    for g in range(32):
        eng_parts[gran_eng[g]].extend(range(4 * g, 4 * g + 4))

    perm = np.empty(B, np.int64)
    w_all = valid.sum(axis=1)  # per-sample fetched blocks
    for c in range(NCORES):
        s0 = c * BL
        order = np.argsort(-w_all[s0 : s0 + BL], kind="stable")
        load = np.zeros(16)
        cap = np.full(16, 8)
        assign = {e: [] for e in range(16)}
        for i in order:
            scaled = (load + w_all[s0 + i]) / rate
            scaled[cap == 0] = np.inf
            e = int(np.argmin(scaled))
            assign[e].append(i)
            load[e] += w_all[s0 + i]
            cap[e] -= 1
        for e in range(16):
            for slot, i in enumerate(assign[e]):
                perm[s0 + eng_parts[e][slot]] = s0 + i

    in_maps = []
    for c in range(NCORES):
        s = slice(c * BL, (c + 1) * BL)
        p = perm[s]
        docs_c = np.zeros((rrows, D), np.float32)
        docs_c[: rows[c]] = de[base[c] : base[c] + rows[c]]
        in_maps.append(
            {
                "rdocs": docs_c,
                "idxs": np.ascontiguousarray(idx_all[p]),
                "q": np.ascontiguousarray(qf[p]),
                "traw": np.ascontiguousarray(traw[p]),
                "mask": np.ascontiguousarray(mask[p].astype(np.uint8)),
            }
        )
    cfg = {"blk": blk, "rrows": rrows, "bufs": bufs, "use_bf16": use_bf16,
           "act_slots": act_slots, "gps_slots": gps_slots}
    return in_maps, cfg


def run(in_maps, cfg=None, trace=False):
    from concourse import bass_utils

    nc = _get_nc(**(cfg or {}))
    return bass_utils.run_bass_kernel_spmd(
        nc, in_maps, list(range(NCORES)), trace=trace
    )


def kernel(query_embeds, doc_embeds, soft_labels, num_docs_per_sample):
    in_maps, cfg = _make_in_maps(
        query_embeds, doc_embeds, soft_labels, num_docs_per_sample
    )
    res = run(in_maps, cfg=cfg)
    tot = sum(float(r["out"][0, 0]) for r in res.results)
    return np.asarray(tot / B, dtype=np.float32)
